# revision 1
# baseline (speedup 1.0000x reference)
"""Continuous Thought Machine kernel for Trainium2 (8 NeuronCores).

Strategy (batch-sharded, per sharding_hint):
  - The dominant FLOPs (206 of 241 GFLOP) are the three feature GEMMs
    kv = LN(x @ W_kv), kh = kv @ W_ak, vh = kv @ W_av over B*S = 32768
    tokens.  These run on the 8 NeuronCores, data-parallel over batch
    (8 batch rows = 4096 tokens per core), via a Bass/Tile GEMM kernel
    launched with bass_utils.run_bass_kernel_spmd.
  - LayerNorm between GEMM1 and GEMM2/3 and the sequential 16-tick
    recurrent loop (34 GFLOP, latency-bound tiny-batch work) run on host
    in fp32, numerically mirroring the reference.
  - Weights are replicated to all cores; x is sharded over batch and
    host-transposed so the contraction dim lands on SBUF partitions.
"""

import numpy as np

B, S, F = 64, 512, 1024
D_MODEL, D_MEM, D_IN, H, D_OUT, N_SYNC, T, H_NLM = 2048, 32, 1024, 16, 1000, 512, 16, 32
DH = D_IN // H
NCORES = 8
BL = B // NCORES          # batch rows per core
NT = BL * S               # tokens per core (4096)

_NC_CACHE = {}
_LAST_EXEC_NS = []


def _build_gemm_nc(a_names, b_name, c_names):
    """Bass/Tile kernel computing C_i = A_i.T @ B for each i.

    A_i: (1024, 1024) fp32, B: (1024, 4096) fp32, C_i: (1024, 4096) fp32.
    Contraction dim (1024) is the DRAM row dim of both operands, so it maps
    straight onto SBUF partitions in 8 chunks of 128 with no transposes.
    """
    import sys
    if "/opt/trn_rl_repo" not in sys.path:
        sys.path.insert(0, "/opt/trn_rl_repo")
    import contextlib

    import concourse.mybir as mybir
    import concourse.tile as tile
    from concourse import bacc

    f32 = mybir.dt.float32
    K, M, N = 1024, 1024, NT
    KC = K // 128

    nc = bacc.Bacc("TRN2", target_bir_lowering=False, debug=False,
                   num_devices=NCORES)
    a_aps = [nc.dram_tensor(an, [K, M], f32, kind="ExternalInput").ap()
             for an in a_names]
    b_ap = nc.dram_tensor(b_name, [K, N], f32, kind="ExternalInput").ap()
    c_aps = [nc.dram_tensor(cn, [M, N], f32, kind="ExternalOutput").ap()
             for cn in c_names]

    with tile.TileContext(nc) as tc:
        with contextlib.ExitStack() as ctx:
            apool = ctx.enter_context(tc.tile_pool(name="apool", bufs=1))
            bpool = ctx.enter_context(tc.tile_pool(name="bpool", bufs=3))
            opool = ctx.enter_context(tc.tile_pool(name="opool", bufs=4))
            pspool = ctx.enter_context(
                tc.tile_pool(name="pspool", bufs=8, space="PSUM"))

            ats = []
            for i in range(len(a_names)):
                at = apool.tile([128, KC, M], f32, tag=f"a{i}")
                nc.sync.dma_start(
                    at[:], a_aps[i].rearrange("(kc p) m -> p kc m", p=128))
                ats.append(at)

            for n0 in range(0, N, 512):
                bt = bpool.tile([128, KC, 512], f32, tag="bt")
                nc.sync.dma_start(
                    bt[:],
                    b_ap[:, n0:n0 + 512].rearrange("(kc p) n -> p kc n", p=128))
                for i in range(len(a_names)):
                    for m0 in range(0, M, 128):
                        ps = pspool.tile([128, 512], f32, tag="ps")
                        for kc in range(KC):
                            nc.tensor.matmul(
                                ps[:],
                                ats[i][:, kc, m0:m0 + 128],
                                bt[:, kc, :],
                                start=(kc == 0),
                                stop=(kc == KC - 1),
                            )
                        ot = opool.tile([128, 512], f32, tag="ot")
                        nc.vector.tensor_copy(ot[:], ps[:])
                        nc.sync.dma_start(c_aps[i][m0:m0 + 128, n0:n0 + 512],
                                          ot[:])
    nc.compile()
    return nc


def _run_gemms(key, a_names, b_name, c_names, in_maps):
    import sys
    if "/opt/trn_rl_repo" not in sys.path:
        sys.path.insert(0, "/opt/trn_rl_repo")
    from concourse.bass_utils import run_bass_kernel_spmd

    if key not in _NC_CACHE:
        _NC_CACHE[key] = _build_gemm_nc(a_names, b_name, c_names)
    nc = _NC_CACHE[key]
    r = run_bass_kernel_spmd(nc, in_maps, list(range(NCORES)))
    if r.exec_time_ns is not None:
        _LAST_EXEC_NS.append(r.exec_time_ns)
    return r.results


def _ln(x, g, b, eps=1e-5):
    m = x.mean(-1, keepdims=True)
    v = ((x - m) ** 2).mean(-1, keepdims=True)
    return ((x - m) / np.sqrt(v + eps)) * g + b


def _gelu_tanh(x):
    # jax.nn.gelu default (approximate=True)
    c = np.float32(np.sqrt(2.0 / np.pi))
    return np.float32(0.5) * x * (np.float32(1.0) +
                                  np.tanh(c * (x + np.float32(0.044715) * x * x * x)))


def kernel(**inputs):
    inp = {k: np.ascontiguousarray(np.asarray(v)) for k, v in inputs.items()}
    x = inp["x"].astype(np.float32, copy=False)

    W_kv, b_kv = inp["W_kv"].astype(np.float32), inp["b_kv"].astype(np.float32)
    g_kv, be_kv = inp["g_kv"].astype(np.float32), inp["be_kv"].astype(np.float32)
    W_ak, b_ak = inp["W_ak"].astype(np.float32), inp["b_ak"].astype(np.float32)
    W_av, b_av = inp["W_av"].astype(np.float32), inp["b_av"].astype(np.float32)

    # ---- Phase A on device: token-sharded GEMMs over 8 cores -------------
    xT = [np.ascontiguousarray(x[c * BL:(c + 1) * BL].reshape(NT, F).T)
          for c in range(NCORES)]
    Wkv_c = np.ascontiguousarray(W_kv)
    try:
        res1 = _run_gemms("g1", ["Wkv"], "xT", ["kvT"],
                          [{"Wkv": Wkv_c, "xT": xT[c]} for c in range(NCORES)])
        kvT = [res1[c]["kvT"] for c in range(NCORES)]
        # host LN (over feature dim; tokens per core stay sharded)
        lnT = []
        for c in range(NCORES):
            kv = kvT[c].T + b_kv                      # (NT, 1024)
            lnT.append(np.ascontiguousarray(_ln(kv, g_kv, be_kv).T))
        Wak_c = np.ascontiguousarray(W_ak)
        Wav_c = np.ascontiguousarray(W_av)
        res2 = _run_gemms(
            "g2", ["Wak", "Wav"], "lnT", ["khT", "vhT"],
            [{"Wak": Wak_c, "Wav": Wav_c, "lnT": lnT[c]} for c in range(NCORES)])
        kh = np.empty((B, S, H, DH), np.float32)
        vh = np.empty((B, S, H, DH), np.float32)
        for c in range(NCORES):
            kh[c * BL:(c + 1) * BL] = (res2[c]["khT"].T + b_ak).reshape(BL, S, H, DH)
            vh[c * BL:(c + 1) * BL] = (res2[c]["vhT"].T + b_av).reshape(BL, S, H, DH)
    except Exception as e:                              # pragma: no cover
        import traceback
        traceback.print_exc()
        print("kernel: device path failed (%r); numpy fallback" % (e,))
        kv = _ln(x.reshape(B * S, F) @ W_kv + b_kv, g_kv, be_kv)
        kh = (kv @ W_ak + b_ak).reshape(B, S, H, DH)
        vh = (kv @ W_av + b_av).reshape(B, S, H, DH)

    # ---- Tick loop on host (fp32, mirrors reference exactly) -------------
    W_q, b_q = inp["W_q"].astype(np.float32), inp["b_q"].astype(np.float32)
    W_aq, b_aq = inp["W_aq"].astype(np.float32), inp["b_aq"].astype(np.float32)
    W_ao, b_ao = inp["W_ao"].astype(np.float32), inp["b_ao"].astype(np.float32)
    W_s1, b_s1 = inp["W_s1"].astype(np.float32), inp["b_s1"].astype(np.float32)
    W_s2, b_s2 = inp["W_s2"].astype(np.float32), inp["b_s2"].astype(np.float32)
    g_s, be_s = inp["g_s"].astype(np.float32), inp["be_s"].astype(np.float32)
    W_n1, b_n1 = inp["W_n1"].astype(np.float32), inp["b_n1"].astype(np.float32)
    W_n2, b_n2 = inp["W_n2"].astype(np.float32), inp["b_n2"].astype(np.float32)
    g_n, be_n = inp["g_n"].astype(np.float32), inp["be_n"].astype(np.float32)
    init_state = inp["init_state"].astype(np.float32)
    init_hist = inp["init_hist"].astype(np.float32)
    W_out, b_out = inp["W_out"].astype(np.float32), inp["b_out"].astype(np.float32)
    idx_la, idx_ra = inp["idx_la"], inp["idx_ra"]
    idx_lo, idx_ro = inp["idx_lo"], inp["idx_ro"]

    rA = np.exp(-np.clip(inp["decay_action"].astype(np.float32), 0.0, 15.0))[None, :]
    rO = np.exp(-np.clip(inp["decay_out"].astype(np.float32), 0.0, 15.0))[None, :]

    act = np.broadcast_to(init_state, (B, D_MODEL)).astype(np.float32).copy()
    hist = np.broadcast_to(init_hist, (B, D_MODEL, D_MEM)).astype(np.float32).copy()
    aO = act[:, idx_lo] * act[:, idx_ro]
    bO = np.ones_like(aO)
    aA = np.zeros((B, N_SYNC), np.float32)
    bA = np.zeros((B, N_SYNC), np.float32)

    inv_sqrt_dh = np.float32(1.0 / np.sqrt(DH))
    preds, nes = [], []
    W_n1_d = np.ascontiguousarray(W_n1)               # (D, M, H)
    for _t in range(T):
        pA = act[:, idx_la] * act[:, idx_ra]
        aA = rA * aA + pA
        bA = rA * bA + np.float32(1.0)
        sync_a = aA / np.sqrt(bA)
        q = sync_a @ W_q + b_q
        qh = (q @ W_aq + b_aq).reshape(B, H, DH)
        scores = np.einsum("bhd,bshd->bhs", qh, kh, optimize=True) * inv_sqrt_dh
        scores = scores - scores.max(-1, keepdims=True)
        e = np.exp(scores)
        attn_w = e / e.sum(-1, keepdims=True)
        attn = np.einsum("bhs,bshd->bhd", attn_w, vh,
                         optimize=True).reshape(B, D_IN) @ W_ao + b_ao
        pre = np.concatenate([attn, act], axis=-1)
        pre_act = _ln(_gelu_tanh(pre @ W_s1 + b_s1) @ W_s2 + b_s2, g_s, be_s)
        hist = np.concatenate([hist[:, :, 1:], pre_act[:, :, None]], axis=-1)
        hmid = np.matmul(hist.transpose(1, 0, 2), W_n1_d)     # (D, B, H)
        hmid = np.maximum(hmid + b_n1[:, None, :], np.float32(0.0))
        act_pre = (hmid * W_n2[:, None, :]).sum(-1).T + b_n2  # (B, D)
        act = _ln(act_pre, g_n, be_n)
        pO = act[:, idx_lo] * act[:, idx_ro]
        aO = rO * aO + pO
        bO = rO * bO + np.float32(1.0)
        pred = (aO / np.sqrt(bO)) @ W_out + b_out
        m = pred.max(-1, keepdims=True)
        lse = m + np.log(np.exp(pred - m).sum(-1, keepdims=True))
        logp = pred - lse
        ne = -(np.exp(logp) * logp).sum(-1) / np.float32(np.log(D_OUT))
        preds.append(pred.astype(np.float32))
        nes.append(ne.astype(np.float32))

    predictions = np.stack(preds, axis=-1)                    # (B, D_OUT, T)
    nes_a = np.stack(nes, axis=0)                             # (T, B)
    certainties = np.stack([nes_a, np.float32(1.0) - nes_a],
                           axis=1).transpose(2, 1, 0)         # (B, 2, T)
    return predictions.astype(np.float32), certainties.astype(np.float32)
